# revision 1
# baseline (speedup 1.0000x reference)
"""MoE FeedForward kernel for Trainium2 (8 NeuronCores).

Strategy:
  - Launch A (data-parallel over tokens): each core LayerNorms its 1024-token
    shard and computes router logits (fp32, exact) on device.
  - Host control plane: softmax/top-2/gate weights + capacity-padded token
    compaction per expert (integer bookkeeping + data shuffling only).
  - Launch B (expert-parallel): core c holds expert c's weights, runs the
    SwiGLU FFN over its compacted tokens in bf16 (fp32 accumulate), applies
    the combine gate on device, and also computes the shared expert for its
    token shard. Host scatters the gated expert outputs back and sums.
"""

import numpy as np
import ml_dtypes

import concourse.bass as bass
import concourse.mybir as mybir
import concourse.tile as tile
from concourse import bacc
from concourse.bass_utils import run_bass_kernel_spmd

F32 = mybir.dt.float32
BF16 = mybir.dt.bfloat16
AF = mybir.ActivationFunctionType
OP = mybir.AluOpType
AX = mybir.AxisListType

NC = 8          # cores / experts
D = 1024        # d_model
DFF = 3072      # routed expert ffn dim
SDFF = 1024     # shared expert ffn dim
T = 8192        # total tokens
TL = T // NC    # tokens per core (launch A)
CAP = 2176      # expert capacity (max measured load 2080 + margin)
LN_EPS = 1e-5

_CACHE = {}


def _bc128(ap):
    """Broadcast a [1, N] DRAM AP across 128 partitions (0-step partition dim)."""
    return bass.AP(tensor=ap.tensor, offset=ap.offset, ap=[[0, 128]] + [list(d) for d in ap.ap[1:]])


# ----------------------------------------------------------------- launch A
def _build_kernel_a():
    nc = bacc.Bacc("TRN2", target_bir_lowering=False, debug=False, num_devices=NC)
    x_tok = nc.dram_tensor("x_tok", [TL, D], F32, kind="ExternalInput")
    x_dT = nc.dram_tensor("x_dT", [D, TL], F32, kind="ExternalInput")
    rwg_T = nc.dram_tensor("rwg_T", [D, NC], F32, kind="ExternalInput")
    c1 = nc.dram_tensor("c1", [1, NC], F32, kind="ExternalInput")
    c0 = nc.dram_tensor("c0", [1, NC], F32, kind="ExternalInput")
    gam = nc.dram_tensor("gam", [1, D], F32, kind="ExternalInput")
    bet = nc.dram_tensor("bet", [1, D], F32, kind="ExternalInput")
    normed = nc.dram_tensor("normed", [TL, D], BF16, kind="ExternalOutput")
    logits = nc.dram_tensor("logits", [TL, NC], F32, kind="ExternalOutput")

    nt = TL // 128
    with tile.TileContext(nc) as tc:
        with tc.tile_pool(name="const", bufs=1) as cp, \
             tc.tile_pool(name="xd", bufs=1) as xdp, \
             tc.tile_pool(name="work", bufs=3) as wp, \
             tc.tile_pool(name="small", bufs=4) as sp, \
             tc.tile_pool(name="ps", bufs=4, space="PSUM") as pp:
            gam_sb = cp.tile([128, D], F32)
            bet_sb = cp.tile([128, D], F32)
            c1_sb = cp.tile([128, NC], F32)
            c0_sb = cp.tile([128, NC], F32)
            nc.gpsimd.dma_start(out=gam_sb[:], in_=_bc128(gam[:]))
            nc.gpsimd.dma_start(out=bet_sb[:], in_=_bc128(bet[:]))
            nc.gpsimd.dma_start(out=c1_sb[:], in_=_bc128(c1[:]))
            nc.gpsimd.dma_start(out=c0_sb[:], in_=_bc128(c0[:]))
            eps_sb = cp.tile([128, 1], F32)
            nc.vector.memset(eps_sb[:], LN_EPS)
            rw_sb = cp.tile([128, 8, NC], F32)
            nc.sync.dma_start(out=rw_sb[:], in_=rwg_T.rearrange("(k p) e -> p k e", p=128))
            xd_sb = xdp.tile([128, 8, TL], F32)
            nc.sync.dma_start(out=xd_sb[:], in_=x_dT.rearrange("(k p) t -> p k t", p=128))

            for tt in range(nt):
                xt = wp.tile([128, D], F32, tag="xt")
                nc.sync.dma_start(out=xt[:], in_=x_tok[tt * 128:(tt + 1) * 128, :])
                mu = sp.tile([128, 1], F32, tag="mu")
                nc.vector.reduce_sum(out=mu[:], in_=xt[:], axis=AX.X)
                nc.vector.tensor_scalar_mul(mu[:], mu[:], 1.0 / D)
                xm = wp.tile([128, D], F32, tag="xm")
                nc.vector.tensor_scalar(out=xm[:], in0=xt[:], scalar1=mu[:], scalar2=None, op0=OP.subtract)
                sq = wp.tile([128, D], F32, tag="sq")
                nc.vector.tensor_tensor(out=sq[:], in0=xm[:], in1=xm[:], op=OP.mult)
                var = sp.tile([128, 1], F32, tag="var")
                nc.vector.reduce_sum(out=var[:], in_=sq[:], axis=AX.X)
                rstd = sp.tile([128, 1], F32, tag="rstd")
                nc.scalar.activation(out=rstd[:], in_=var[:], func=AF.Sqrt, scale=1.0 / D, bias=eps_sb[:])
                nc.vector.reciprocal(out=rstd[:], in_=rstd[:])
                # normed = (x-mu)*rstd*gamma + beta   (bf16 out)
                nrm_f = wp.tile([128, D], F32, tag="nrm_f")
                nc.vector.tensor_scalar_mul(nrm_f[:], xm[:], rstd[:])
                nc.vector.tensor_tensor(out=nrm_f[:], in0=nrm_f[:], in1=gam_sb[:], op=OP.mult)
                nrm_b = wp.tile([128, D], BF16, tag="nrm_b")
                nc.vector.tensor_tensor(out=nrm_b[:], in0=nrm_f[:], in1=bet_sb[:], op=OP.add)
                nc.sync.dma_start(out=normed[tt * 128:(tt + 1) * 128, :], in_=nrm_b[:])
                # router logits from raw x:  rstd*(x@ (rw*gamma).T - mu*c1) + c0
                praw = pp.tile([128, NC], F32, space="PSUM", tag="praw")
                for k in range(8):
                    nc.tensor.matmul(out=praw[:], lhsT=xd_sb[:, k, tt * 128:(tt + 1) * 128],
                                     rhs=rw_sb[:, k, :], start=(k == 0), stop=(k == 7))
                lg = sp.tile([128, NC], F32, tag="lg")
                nc.vector.tensor_scalar(out=lg[:], in0=c1_sb[:], scalar1=mu[:], scalar2=None, op0=OP.mult)
                nc.vector.tensor_tensor(out=lg[:], in0=praw[:], in1=lg[:], op=OP.subtract)
                nc.vector.tensor_scalar_mul(lg[:], lg[:], rstd[:])
                nc.vector.tensor_tensor(out=lg[:], in0=lg[:], in1=c0_sb[:], op=OP.add)
                nc.sync.dma_start(out=logits[tt * 128:(tt + 1) * 128, :], in_=lg[:])
    nc.compile()
    return nc


# ----------------------------------------------------------------- launch B
def _ffn(nc, tc, ctx_pools, xT_dram, gup_sb, down_sb, n_ftiles, blocks, out_dram, gates_sb):
    """SwiGLU FFN: out.T[d, tok] = down @ (silu(gate) * up); optionally gate-scaled.

    Weights (gup_sb, down_sb) are SBUF-resident; token tiles stream per block.
    """
    hp, xp, pg_p, py_p, ev_p = ctx_pools
    off = 0
    for nbw in blocks:
        xT_sb = []
        for k in range(8):
            x = xp.tile([128, nbw], BF16, tag=f"xb{k}")
            # gpsimd queue: keeps token tiles out of the weight-load FIFO on sync
            nc.gpsimd.dma_start(out=x[:], in_=xT_dram[k * 128:(k + 1) * 128, off:off + nbw])
            xT_sb.append(x)
        hts = []
        for fi in range(n_ftiles):
            pg = pg_p.tile([128, nbw], F32, space="PSUM", tag="pg")
            pu = pg_p.tile([128, nbw], F32, space="PSUM", tag="pu")
            for k in range(8):
                nc.tensor.matmul(out=pg[:], lhsT=gup_sb[k][:, fi * 128:(fi + 1) * 128],
                                 rhs=xT_sb[k][:], start=(k == 0), stop=(k == 7))
            for k in range(8):
                nc.tensor.matmul(out=pu[:], lhsT=gup_sb[k][:, (n_ftiles + fi) * 128:(n_ftiles + fi + 1) * 128],
                                 rhs=xT_sb[k][:], start=(k == 0), stop=(k == 7))
            sil = ev_p.tile([128, nbw], BF16, tag="sil")
            nc.scalar.activation(out=sil[:], in_=pg[:], func=AF.Silu)
            ht = hp.tile([128, nbw], BF16, tag=f"h{fi}")
            nc.vector.tensor_tensor(out=ht[:], in0=sil[:], in1=pu[:], op=OP.mult)
            hts.append(ht)
        for m in range(8):
            py = py_p.tile([128, nbw], F32, space="PSUM", tag="py")
            for kf in range(n_ftiles):
                nc.tensor.matmul(out=py[:], lhsT=down_sb[kf][:, m * 128:(m + 1) * 128],
                                 rhs=hts[kf][:], start=(kf == 0), stop=(kf == n_ftiles - 1))
            yo = ev_p.tile([128, nbw], F32, tag="yo")
            if gates_sb is not None:
                nc.vector.tensor_tensor(out=yo[:], in0=py[:], in1=gates_sb[:, off:off + nbw], op=OP.mult)
            else:
                nc.vector.tensor_copy(out=yo[:], in_=py[:])
            nc.sync.dma_start(out=out_dram[m * 128:(m + 1) * 128, off:off + nbw], in_=yo[:])
        off += nbw


def _build_kernel_b():
    nc = bacc.Bacc("TRN2", target_bir_lowering=False, debug=False, num_devices=NC)
    xcT = nc.dram_tensor("xcT", [D, CAP], BF16, kind="ExternalInput")
    gupT = nc.dram_tensor("gupT", [D, 2 * DFF], BF16, kind="ExternalInput")
    downT = nc.dram_tensor("downT", [DFF, D], BF16, kind="ExternalInput")
    gates = nc.dram_tensor("gates", [1, CAP], F32, kind="ExternalInput")
    xsT = nc.dram_tensor("xsT", [D, TL], BF16, kind="ExternalInput")
    sgupT = nc.dram_tensor("sgupT", [D, 2 * SDFF], BF16, kind="ExternalInput")
    sdownT = nc.dram_tensor("sdownT", [SDFF, D], BF16, kind="ExternalInput")
    yT = nc.dram_tensor("yT", [D, CAP], F32, kind="ExternalOutput")
    ysT = nc.dram_tensor("ysT", [D, TL], F32, kind="ExternalOutput")

    with tile.TileContext(nc) as tc:
        with tc.tile_pool(name="h", bufs=1) as hp, \
             tc.tile_pool(name="xb", bufs=2) as xp, \
             tc.tile_pool(name="pg", bufs=3, space="PSUM") as pg_p, \
             tc.tile_pool(name="py", bufs=2, space="PSUM") as py_p, \
             tc.tile_pool(name="ev", bufs=4) as ev_p:
            # routed expert: all weights SBUF-resident, token tiles streamed
            with tc.tile_pool(name="wts1", bufs=1) as wtp:
                gat_sb = wtp.tile([128, CAP], F32, tag="gat")
                nc.gpsimd.dma_start(out=gat_sb[:], in_=_bc128(gates[:]))
                gup_sb = [wtp.tile([128, 2 * DFF], BF16, tag=f"gup{k}", name=f"gup{k}")
                          for k in range(8)]
                # column-chunked, all k first: first matmuls unblock after 8 small DMAs
                for h in range(8):
                    for k in range(8):
                        nc.sync.dma_start(
                            out=gup_sb[k][:, h * DFF // 4:(h + 1) * DFF // 4],
                            in_=gupT[k * 128:(k + 1) * 128, h * DFF // 4:(h + 1) * DFF // 4])
                down_sb = []
                for kf in range(DFF // 128):
                    t = wtp.tile([128, D], BF16, tag=f"dn{kf}")
                    nc.sync.dma_start(out=t[:], in_=downT[kf * 128:(kf + 1) * 128, :])
                    down_sb.append(t)
                blocks = [512] * (CAP // 512) + ([CAP % 512] if CAP % 512 else [])
                _ffn(nc, tc, (hp, xp, pg_p, py_p, ev_p), xcT, gup_sb, down_sb,
                     DFF // 128, blocks, yT, gat_sb)
            # shared expert on my token shard
            with tc.tile_pool(name="wts2", bufs=1) as wtp:
                sgup_sb = []
                for k in range(8):
                    g = wtp.tile([128, 2 * SDFF], BF16, tag=f"sgup{k}")
                    nc.sync.dma_start(out=g[:], in_=sgupT[k * 128:(k + 1) * 128, :])
                    sgup_sb.append(g)
                sdn_sb = []
                for kf in range(SDFF // 128):
                    t = wtp.tile([128, D], BF16, tag=f"sdn{kf}")
                    nc.sync.dma_start(out=t[:], in_=sdownT[kf * 128:(kf + 1) * 128, :])
                    sdn_sb.append(t)
                _ffn(nc, tc, (hp, xp, pg_p, py_p, ev_p), xsT, sgup_sb, sdn_sb,
                     SDFF // 128, [512, 512], ysT, None)
    nc.compile()
    return nc


def _get(name, builder):
    if name not in _CACHE:
        _CACHE[name] = builder()
    return _CACHE[name]


def _to_bf16(a):
    return np.ascontiguousarray(a.astype(ml_dtypes.bfloat16))


def kernel(x, ln_gamma, ln_beta, router_w, gate_up_w, down_w,
           shared_gate_up_w, shared_down_w, _profile=None):
    x = np.asarray(x, np.float32)
    B, S, _ = x.shape
    xt = np.ascontiguousarray(x.reshape(T, D))
    rwg = np.ascontiguousarray((router_w * ln_gamma[None, :]).T.astype(np.float32))  # [D, E]
    c1 = (router_w @ ln_gamma).astype(np.float32).reshape(1, NC)
    c0 = (router_w @ ln_beta).astype(np.float32).reshape(1, NC)

    # ---- launch A: LayerNorm + router logits (device)
    nc_a = _get("a", _build_kernel_a)
    in_maps = []
    for c in range(NC):
        sh = xt[c * TL:(c + 1) * TL]
        in_maps.append(dict(
            x_tok=np.ascontiguousarray(sh),
            x_dT=np.ascontiguousarray(sh.T),
            rwg_T=rwg, c1=c1, c0=c0,
            gam=ln_gamma.reshape(1, D).astype(np.float32),
            bet=ln_beta.reshape(1, D).astype(np.float32),
        ))
    kw = {k: v for k, v in (_profile or {}).items() if k in ("trace", "tmpdir")}
    kwa = dict(kw)
    if "tmpdir" in kwa:
        kwa["tmpdir"] = kwa["tmpdir"] + "_a"
    res_a = run_bass_kernel_spmd(nc_a, in_maps, list(range(NC)), **kwa)
    normed = np.concatenate([res_a.results[c]["normed"] for c in range(NC)], axis=0)
    logits = np.concatenate([res_a.results[c]["logits"] for c in range(NC)], axis=0)
    if _profile is not None:
        _profile["exec_a"] = res_a.exec_time_ns

    # ---- host control plane: softmax / top-2 / capacity compaction
    lg = logits.astype(np.float32)
    p = np.exp(lg - lg.max(-1, keepdims=True))
    p /= p.sum(-1, keepdims=True)
    order = np.argsort(-p, axis=-1, kind="stable")
    top2 = order[:, :2]
    pv = np.take_along_axis(p, top2, axis=1)
    g = np.exp(pv - pv.max(-1, keepdims=True))
    g /= g.sum(-1, keepdims=True)

    normed_f = normed.astype(np.float32)
    idxs, gvals = [], []
    for e in range(NC):
        hit = (top2 == e)
        ide = np.where(hit.any(axis=1))[0]
        ge = np.where(hit[ide, 0], g[ide, 0], g[ide, 1]).astype(np.float32)
        assert len(ide) <= CAP, f"expert {e} overflow: {len(ide)}"
        idxs.append(ide)
        gvals.append(ge)

    # ---- launch B: expert FFNs (device, expert-parallel) + shared expert
    nc_b = _get("b", _build_kernel_b)
    sgupT = _to_bf16(shared_gate_up_w.T)
    sdownT = _to_bf16(shared_down_w.T)
    in_maps = []
    for c in range(NC):
        ide, ge = idxs[c], gvals[c]
        xc = np.zeros((D, CAP), ml_dtypes.bfloat16)
        xc[:, :len(ide)] = _to_bf16(normed_f[ide].T)
        gr = np.zeros((1, CAP), np.float32)
        gr[0, :len(ide)] = ge
        in_maps.append(dict(
            xcT=xc,
            gupT=_to_bf16(gate_up_w[c].T),
            downT=_to_bf16(down_w[c].T),
            gates=gr,
            xsT=np.ascontiguousarray(normed[c * TL:(c + 1) * TL].T),
            sgupT=sgupT, sdownT=sdownT,
        ))
    kwb = dict(kw)
    if "tmpdir" in kwb:
        kwb["tmpdir"] = kwb["tmpdir"] + "_b"
    res_b = run_bass_kernel_spmd(nc_b, in_maps, list(range(NC)), **kwb)
    if _profile is not None:
        _profile["exec_b"] = res_b.exec_time_ns

    # ---- host: scatter-add combine (data movement + elementwise add)
    out = np.zeros((T, D), np.float32)
    for c in range(NC):
        ide = idxs[c]
        out[ide] += res_b.results[c]["yT"][:, :len(ide)].T
        out[c * TL:(c + 1) * TL] += res_b.results[c]["ysT"].T
    return out.reshape(B, S, D)



# revision 8
# speedup vs baseline: 1.0625x; 1.0625x over previous
"""MoE FeedForward kernel for Trainium2 (8 NeuronCores).

Strategy:
  - Launch A (data-parallel over tokens): each core LayerNorms its 1024-token
    shard (scalar-engine accumulate for mean/var, DVE kept nearly idle),
    computes router logits (fp32, exact), transposes the normalized tokens
    on the PE, and runs the SHARED expert FFN over its shard (this fills the
    otherwise-idle tensor engine in A and removes the SBUF weight-pool
    transition stall from launch B).
  - Host control plane: softmax/top-2/gate weights + capacity-padded token
    compaction per expert (integer bookkeeping + data shuffling only).
  - Launch B (expert-parallel): core c holds expert c's weights, runs the
    routed SwiGLU FFN over its compacted tokens in bf16 (fp32 accumulate)
    and applies the combine gate on device. Capacity is derived from the
    actual max expert load at runtime (compiled per capacity, cached).
    Host scatters the gated expert outputs back and sums.
"""

import math

import numpy as np
import ml_dtypes

import concourse.bass as bass
import concourse.mybir as mybir
import concourse.tile as tile
from concourse import bacc
from concourse.bass_utils import run_bass_kernel_spmd
from concourse.masks import make_identity

F32 = mybir.dt.float32
BF16 = mybir.dt.bfloat16
AF = mybir.ActivationFunctionType
OP = mybir.AluOpType
AX = mybir.AxisListType

NC = 8          # cores / experts
D = 1024        # d_model
DFF = 3072      # routed expert ffn dim
SDFF = 1024     # shared expert ffn dim
T = 8192        # total tokens
TL = T // NC    # tokens per core (launch A)
LN_EPS = 1e-5

_CACHE = {}


def _bc128(ap):
    """Broadcast a [1, N] DRAM AP across 128 partitions (0-step partition dim)."""
    return bass.AP(tensor=ap.tensor, offset=ap.offset, ap=[[0, 128]] + [list(d) for d in ap.ap[1:]])


# ----------------------------------------------------------------- launch A
def _build_kernel_a(affine):
    nc = bacc.Bacc("TRN2", target_bir_lowering=False, debug=False, num_devices=NC)
    x_tok = nc.dram_tensor("x_tok", [TL, D], F32, kind="ExternalInput")
    x_dT = nc.dram_tensor("x_dT", [D, TL], F32, kind="ExternalInput")
    rwg_T = nc.dram_tensor("rwg_T", [D, NC], F32, kind="ExternalInput")
    c1 = nc.dram_tensor("c1", [1, NC], F32, kind="ExternalInput")
    c0 = nc.dram_tensor("c0", [1, NC], F32, kind="ExternalInput")
    sgupT = nc.dram_tensor("sgupT", [D, 2 * SDFF], BF16, kind="ExternalInput")
    sdownT = nc.dram_tensor("sdownT", [SDFF, D], BF16, kind="ExternalInput")
    if affine:
        gam = nc.dram_tensor("gam", [1, D], F32, kind="ExternalInput")
        bet = nc.dram_tensor("bet", [1, D], F32, kind="ExternalInput")
    zT = nc.dram_tensor("zT", [D, TL], BF16, kind="ExternalOutput")
    logits = nc.dram_tensor("logits", [TL, NC], F32, kind="ExternalOutput")
    ysT = nc.dram_tensor("ysT", [D, TL], F32, kind="ExternalOutput")

    nt = TL // 128
    with tile.TileContext(nc) as tc:
        with tc.tile_pool(name="const", bufs=1) as cp, \
             tc.tile_pool(name="xd", bufs=1) as xdp, \
             tc.tile_pool(name="work", bufs=2) as wp, \
             tc.tile_pool(name="small", bufs=4) as sp, \
             tc.tile_pool(name="swts", bufs=1) as swp, \
             tc.tile_pool(name="h", bufs=1) as hp, \
             tc.tile_pool(name="ev", bufs=4) as ev_p, \
             tc.tile_pool(name="praw", bufs=1, space="PSUM") as pp, \
             tc.tile_pool(name="tp", bufs=1, space="PSUM") as tpp, \
             tc.tile_pool(name="pg", bufs=2, space="PSUM") as pg_p, \
             tc.tile_pool(name="py", bufs=2, space="PSUM") as py_p:
            # constants
            c1_sb = cp.tile([128, NC], F32)
            c0_sb = cp.tile([128, NC], F32)
            nc.gpsimd.dma_start(out=c1_sb[:], in_=_bc128(c1[:]))
            nc.gpsimd.dma_start(out=c0_sb[:], in_=_bc128(c0[:]))
            eps_sb = cp.tile([128, 1], F32)
            nc.vector.memset(eps_sb[:], LN_EPS)
            id_sb = cp.tile([128, 128], BF16)
            make_identity(nc, id_sb[:])
            rw_sb = cp.tile([128, 8, NC], F32)
            nc.sync.dma_start(out=rw_sb[:], in_=rwg_T.rearrange("(k p) e -> p k e", p=128))
            if affine:
                gam_sb = cp.tile([128, D], F32)
                bet_sb = cp.tile([128, D], F32)
                nc.gpsimd.dma_start(out=gam_sb[:], in_=_bc128(gam[:]))
                nc.gpsimd.dma_start(out=bet_sb[:], in_=_bc128(bet[:]))
            xd_sb = xdp.tile([128, 8, TL], F32)
            zT_sb = xdp.tile([128, 8, TL], BF16)
            xd_r = x_dT.rearrange("(k p) t -> p k t", p=128)
            zT_r = zT.rearrange("(k p) t -> p k t", p=128)
            # sync-queue order: x_dT tiles 0-3 (unblock router), shared gate_up
            # weights (needed ~20us in), x_dT tiles 4-7, shared down weights.
            for tt in range(4):
                sl = slice(tt * 128, (tt + 1) * 128)
                nc.sync.dma_start(out=xd_sb[:, :, sl], in_=xd_r[:, :, sl])
            sgup_sb = []
            for k in range(8):
                g = swp.tile([128, 2 * SDFF], BF16, tag=f"sgup{k}")
                nc.sync.dma_start(out=g[:], in_=sgupT[k * 128:(k + 1) * 128, :])
                sgup_sb.append(g)
            for tt in range(4, 8):
                sl = slice(tt * 128, (tt + 1) * 128)
                nc.sync.dma_start(out=xd_sb[:, :, sl], in_=xd_r[:, :, sl])
            sdn_sb = []
            for kf in range(SDFF // 128):
                t = swp.tile([128, D], BF16, tag=f"sdn{kf}")
                nc.sync.dma_start(out=t[:], in_=sdownT[kf * 128:(kf + 1) * 128, :])
                sdn_sb.append(t)

            def shared_block(b):
                off = b * 512
                hts = []
                for fi in range(SDFF // 128):
                    pg = pg_p.tile([128, 512], F32, space="PSUM", tag="pg")
                    pu = pg_p.tile([128, 512], F32, space="PSUM", tag="pu")
                    for k in range(8):
                        nc.tensor.matmul(out=pg[:], lhsT=sgup_sb[k][:, fi * 128:(fi + 1) * 128],
                                         rhs=zT_sb[:, k, off:off + 512], start=(k == 0), stop=(k == 7))
                    for k in range(8):
                        nc.tensor.matmul(out=pu[:], lhsT=sgup_sb[k][:, (SDFF // 128 + fi) * 128:(SDFF // 128 + fi + 1) * 128],
                                         rhs=zT_sb[:, k, off:off + 512], start=(k == 0), stop=(k == 7))
                    sil = ev_p.tile([128, 512], BF16, tag="sil")
                    nc.scalar.activation(out=sil[:], in_=pg[:], func=AF.Silu)
                    ht = hp.tile([128, 512], BF16, tag=f"sh{fi}")
                    nc.vector.tensor_tensor(out=ht[:], in0=sil[:], in1=pu[:], op=OP.mult)
                    hts.append(ht)
                for m in range(8):
                    py = py_p.tile([128, 512], F32, space="PSUM", tag="py")
                    for kf in range(SDFF // 128):
                        nc.tensor.matmul(out=py[:], lhsT=sdn_sb[kf][:, m * 128:(m + 1) * 128],
                                         rhs=hts[kf][:], start=(kf == 0), stop=(kf == SDFF // 128 - 1))
                    yo = ev_p.tile([128, 512], F32, tag="yo")
                    nc.vector.tensor_copy(out=yo[:], in_=py[:])
                    nc.sync.dma_start(out=ysT[m * 128:(m + 1) * 128, off:off + 512], in_=yo[:])

            for tt in range(nt):
                sl = slice(tt * 128, (tt + 1) * 128)
                xt = wp.tile([128, D], F32, tag="xt")
                nc.gpsimd.dma_start(out=xt[:], in_=x_tok[sl, :])
                # mean / var via scalar-engine accumulate
                scr = wp.tile([128, D], BF16, tag="scr")
                sx = sp.tile([128, 1], F32, tag="sx")
                nc.scalar.activation(out=scr[:], in_=xt[:], func=AF.Copy, accum_out=sx[:])
                scr2 = wp.tile([128, D], BF16, tag="scr2")
                ssq = sp.tile([128, 1], F32, tag="ssq")
                nc.scalar.activation(out=scr2[:], in_=xt[:], func=AF.Square, accum_out=ssq[:])
                mu = sp.tile([128, 1], F32, tag="mu")
                nc.vector.tensor_scalar_mul(mu[:], sx[:], 1.0 / D)
                msq = sp.tile([128, 1], F32, tag="msq")
                nc.vector.tensor_tensor(out=msq[:], in0=mu[:], in1=mu[:], op=OP.mult)
                bt = sp.tile([128, 1], F32, tag="bt")
                nc.vector.tensor_tensor(out=bt[:], in0=eps_sb[:], in1=msq[:], op=OP.subtract)
                rstd = sp.tile([128, 1], F32, tag="rstd")
                nc.scalar.activation(out=rstd[:], in_=ssq[:], func=AF.Sqrt, scale=1.0 / D, bias=bt[:])
                nc.vector.reciprocal(out=rstd[:], in_=rstd[:])
                mrs = sp.tile([128, 1], F32, tag="mrs")
                nc.vector.tensor_tensor(out=mrs[:], in0=mu[:], in1=rstd[:], op=OP.mult)
                zb = sp.tile([128, 1], F32, tag="zb")
                nc.vector.tensor_scalar_mul(zb[:], mrs[:], -1.0)
                # router logits from raw x:  rstd*(x@rwg - mu*c1) + c0
                praw = pp.tile([128, NC], F32, space="PSUM", tag="praw")
                for k in range(8):
                    nc.tensor.matmul(out=praw[:], lhsT=xd_sb[:, k, sl],
                                     rhs=rw_sb[:, k, :], start=(k == 0), stop=(k == 7))
                lg = sp.tile([128, NC], F32, tag="lg")
                nc.vector.tensor_scalar(out=lg[:], in0=c1_sb[:], scalar1=mu[:], scalar2=None, op0=OP.mult)
                nc.vector.tensor_tensor(out=lg[:], in0=praw[:], in1=lg[:], op=OP.subtract)
                nc.vector.tensor_scalar_mul(lg[:], lg[:], rstd[:])
                nc.vector.tensor_tensor(out=lg[:], in0=lg[:], in1=c0_sb[:], op=OP.add)
                nc.sync.dma_start(out=logits[sl, :], in_=lg[:])
                # z = (x - mu) * rstd   (bf16; optional affine)
                if affine:
                    zf = wp.tile([128, D], F32, tag="zf")
                    nc.scalar.activation(out=zf[:], in_=xt[:], func=AF.Identity, scale=rstd[:], bias=zb[:])
                    zg = wp.tile([128, D], F32, tag="zg")
                    nc.vector.tensor_tensor(out=zg[:], in0=zf[:], in1=gam_sb[:], op=OP.mult)
                    z = wp.tile([128, D], BF16, tag="z")
                    nc.vector.tensor_tensor(out=z[:], in0=zg[:], in1=bet_sb[:], op=OP.add)
                else:
                    z = wp.tile([128, D], BF16, tag="z")
                    nc.scalar.activation(out=z[:], in_=xt[:], func=AF.Identity, scale=rstd[:], bias=zb[:])
                # transpose z into zT_sb (PE transpose via identity)
                for k in range(8):
                    pt = tpp.tile([128, 128], BF16, space="PSUM", tag="pt")
                    nc.tensor.transpose(pt[:], z[:, k * 128:(k + 1) * 128], id_sb[:])
                    nc.vector.tensor_copy(out=zT_sb[:, k, sl], in_=pt[:])
                nc.sync.dma_start(out=zT_r[:, :, sl], in_=zT_sb[:, :, sl])
                if tt == 3:
                    shared_block(0)
            shared_block(1)
    nc.compile()
    return nc


# ----------------------------------------------------------------- launch B
def _build_kernel_b(cap, bw, nb):
    nc = bacc.Bacc("TRN2", target_bir_lowering=False, debug=False, num_devices=NC)
    xcT = nc.dram_tensor("xcT", [D, cap], BF16, kind="ExternalInput")
    gupT = nc.dram_tensor("gupT", [D, 2 * DFF], BF16, kind="ExternalInput")
    downT = nc.dram_tensor("downT", [DFF, D], BF16, kind="ExternalInput")
    gates = nc.dram_tensor("gates", [1, cap], F32, kind="ExternalInput")
    yT = nc.dram_tensor("yT", [D, cap], F32, kind="ExternalOutput")

    nf = DFF // 128
    with tile.TileContext(nc) as tc:
        with tc.tile_pool(name="h", bufs=1) as hp, \
             tc.tile_pool(name="xb", bufs=2) as xp, \
             tc.tile_pool(name="pg", bufs=3, space="PSUM") as pg_p, \
             tc.tile_pool(name="py", bufs=2, space="PSUM") as py_p, \
             tc.tile_pool(name="ev", bufs=4) as ev_p, \
             tc.tile_pool(name="wts", bufs=1) as wtp:
            gat_sb = wtp.tile([128, cap], F32, tag="gat")
            nc.gpsimd.dma_start(out=gat_sb[:], in_=_bc128(gates[:]))
            gup_sb = [wtp.tile([128, 2 * DFF], BF16, tag=f"gup{k}", name=f"gup{k}")
                      for k in range(8)]
            # column-chunked, all k first: first matmuls unblock after 8 small DMAs
            for h in range(8):
                for k in range(8):
                    nc.sync.dma_start(
                        out=gup_sb[k][:, h * DFF // 4:(h + 1) * DFF // 4],
                        in_=gupT[k * 128:(k + 1) * 128, h * DFF // 4:(h + 1) * DFF // 4])
            down_sb = []
            for kf in range(nf):
                t = wtp.tile([128, D], BF16, tag=f"dn{kf}")
                nc.sync.dma_start(out=t[:], in_=downT[kf * 128:(kf + 1) * 128, :])
                down_sb.append(t)
            off = 0
            for b in range(nb):
                xT_sb = []
                for k in range(8):
                    x = xp.tile([128, bw], BF16, tag=f"xb{k}")
                    nc.gpsimd.dma_start(out=x[:], in_=xcT[k * 128:(k + 1) * 128, off:off + bw])
                    xT_sb.append(x)
                hts = []
                for fi in range(nf):
                    pg = pg_p.tile([128, bw], F32, space="PSUM", tag="pg")
                    pu = pg_p.tile([128, bw], F32, space="PSUM", tag="pu")
                    for k in range(8):
                        nc.tensor.matmul(out=pg[:], lhsT=gup_sb[k][:, fi * 128:(fi + 1) * 128],
                                         rhs=xT_sb[k][:], start=(k == 0), stop=(k == 7))
                    for k in range(8):
                        nc.tensor.matmul(out=pu[:], lhsT=gup_sb[k][:, (nf + fi) * 128:(nf + fi + 1) * 128],
                                         rhs=xT_sb[k][:], start=(k == 0), stop=(k == 7))
                    sil = ev_p.tile([128, bw], BF16, tag="sil")
                    nc.scalar.activation(out=sil[:], in_=pg[:], func=AF.Silu)
                    ht = hp.tile([128, bw], BF16, tag=f"h{fi}")
                    nc.vector.tensor_tensor(out=ht[:], in0=sil[:], in1=pu[:], op=OP.mult)
                    hts.append(ht)
                for m in range(8):
                    py = py_p.tile([128, bw], F32, space="PSUM", tag="py")
                    for kf in range(nf):
                        nc.tensor.matmul(out=py[:], lhsT=down_sb[kf][:, m * 128:(m + 1) * 128],
                                         rhs=hts[kf][:], start=(kf == 0), stop=(kf == nf - 1))
                    yo = ev_p.tile([128, bw], F32, tag="yo")
                    nc.vector.tensor_tensor(out=yo[:], in0=py[:], in1=gat_sb[:, off:off + bw], op=OP.mult)
                    nc.sync.dma_start(out=yT[m * 128:(m + 1) * 128, off:off + bw], in_=yo[:])
                off += bw
    nc.compile()
    return nc


def _get(key, builder, *args):
    if key not in _CACHE:
        _CACHE[key] = builder(*args)
    return _CACHE[key]


def _to_bf16(a):
    return np.ascontiguousarray(a.astype(ml_dtypes.bfloat16))


def kernel(x, ln_gamma, ln_beta, router_w, gate_up_w, down_w,
           shared_gate_up_w, shared_down_w, _profile=None):
    x = np.asarray(x, np.float32)
    B, S, _ = x.shape
    xt = np.ascontiguousarray(x.reshape(T, D))
    ln_gamma = np.asarray(ln_gamma, np.float32)
    ln_beta = np.asarray(ln_beta, np.float32)
    affine = bool(np.any(ln_gamma != 1.0) or np.any(ln_beta != 0.0))
    rwg = np.ascontiguousarray((router_w * ln_gamma[None, :]).T.astype(np.float32))  # [D, E]
    c1 = (router_w @ ln_gamma).astype(np.float32).reshape(1, NC)
    c0 = (router_w @ ln_beta).astype(np.float32).reshape(1, NC)
    sgupT = _to_bf16(shared_gate_up_w.T)
    sdownT = _to_bf16(shared_down_w.T)

    # ---- launch A: LayerNorm + router logits + shared expert (device)
    nc_a = _get(("a", affine), _build_kernel_a, affine)
    in_maps = []
    for c in range(NC):
        sh = xt[c * TL:(c + 1) * TL]
        m = dict(
            x_tok=np.ascontiguousarray(sh),
            x_dT=np.ascontiguousarray(sh.T),
            rwg_T=rwg, c1=c1, c0=c0,
            sgupT=sgupT, sdownT=sdownT,
        )
        if affine:
            m["gam"] = ln_gamma.reshape(1, D)
            m["bet"] = ln_beta.reshape(1, D)
        in_maps.append(m)
    kw = {k: v for k, v in (_profile or {}).items() if k in ("trace", "tmpdir")}
    kwa = dict(kw)
    if "tmpdir" in kwa:
        kwa["tmpdir"] = kwa["tmpdir"] + "_a"
    res_a = run_bass_kernel_spmd(nc_a, in_maps, list(range(NC)), **kwa)
    zT_all = np.concatenate([res_a.results[c]["zT"] for c in range(NC)], axis=1)  # [D, T] bf16
    logits = np.concatenate([res_a.results[c]["logits"] for c in range(NC)], axis=0)
    if _profile is not None:
        _profile["exec_a"] = res_a.exec_time_ns

    # ---- host control plane: softmax / top-2 / capacity compaction
    lg = logits.astype(np.float32)
    p = np.exp(lg - lg.max(-1, keepdims=True))
    p /= p.sum(-1, keepdims=True)
    order = np.argsort(-p, axis=-1, kind="stable")
    top2 = order[:, :2]
    pv = np.take_along_axis(p, top2, axis=1)
    g = np.exp(pv - pv.max(-1, keepdims=True))
    g /= g.sum(-1, keepdims=True)

    idxs, gvals = [], []
    for e in range(NC):
        hit = (top2 == e)
        ide = np.where(hit.any(axis=1))[0]
        ge = np.where(hit[ide, 0], g[ide, 0], g[ide, 1]).astype(np.float32)
        idxs.append(ide)
        gvals.append(ge)
    max_load = max(len(ide) for ide in idxs)
    nb = max(1, math.ceil(max_load / 512))
    bw = math.ceil(max_load / nb / 16) * 16
    cap = nb * bw

    # ---- launch B: routed expert FFNs (device, expert-parallel)
    nc_b = _get(("b", cap, bw, nb), _build_kernel_b, cap, bw, nb)
    in_maps = []
    for c in range(NC):
        ide, ge = idxs[c], gvals[c]
        xc = np.zeros((D, cap), ml_dtypes.bfloat16)
        xc[:, :len(ide)] = zT_all[:, ide]
        gr = np.zeros((1, cap), np.float32)
        gr[0, :len(ide)] = ge
        in_maps.append(dict(
            xcT=xc,
            gupT=_to_bf16(gate_up_w[c].T),
            downT=_to_bf16(down_w[c].T),
            gates=gr,
        ))
    kwb = dict(kw)
    if "tmpdir" in kwb:
        kwb["tmpdir"] = kwb["tmpdir"] + "_b"
    res_b = run_bass_kernel_spmd(nc_b, in_maps, list(range(NC)), **kwb)
    if _profile is not None:
        _profile["exec_b"] = res_b.exec_time_ns

    # ---- host: scatter-add combine (data movement + elementwise add)
    out = np.zeros((T, D), np.float32)
    for c in range(NC):
        ide = idxs[c]
        out[ide] += res_b.results[c]["yT"][:, :len(ide)].T
        out[c * TL:(c + 1) * TL] += res_a.results[c]["ysT"].T
    return out.reshape(B, S, D)


# revision 13
# speedup vs baseline: 1.0708x; 1.0078x over previous
"""MoE FeedForward kernel for Trainium2 (8 NeuronCores).

Strategy:
  - Launch A (data-parallel over tokens): each core computes router logits
    (fp32, exact, E-major with a fused ones-column for the token mean),
    sum-of-squares via a bf16 ones-matmul, per-token LayerNorm stats as
    [1, T] rows, broadcasts them with PE outer products, produces the
    normalized tokens DIRECTLY in transposed (d-major) layout via DVE, and
    runs the SHARED expert FFN over its shard (filling the otherwise-idle
    tensor engine).
  - Host control plane: softmax/top-2/gate weights + capacity-padded token
    compaction per expert (integer bookkeeping + data shuffling only).
  - Launch B (expert-parallel): core c holds expert c's weights, runs the
    routed SwiGLU FFN over its compacted tokens in bf16 (fp32 accumulate)
    and applies the combine gate on device. Capacity is derived from the
    actual max expert load at runtime (compiled per capacity, cached).
    Host scatters the gated expert outputs back and sums.
"""

import math

import numpy as np
import ml_dtypes

import concourse.bass as bass
import concourse.mybir as mybir
import concourse.tile as tile
from concourse import bacc
from concourse.bass_utils import run_bass_kernel_spmd

F32 = mybir.dt.float32
BF16 = mybir.dt.bfloat16
AF = mybir.ActivationFunctionType
OP = mybir.AluOpType
AX = mybir.AxisListType

NC = 8          # cores / experts
D = 1024        # d_model
DFF = 3072      # routed expert ffn dim
SDFF = 1024     # shared expert ffn dim
T = 8192        # total tokens
TL = T // NC    # tokens per core (launch A)
LN_EPS = 1e-5

_CACHE = {}


def _bc128(ap):
    """Broadcast a [1, N] DRAM AP across 128 partitions (0-step partition dim)."""
    return bass.AP(tensor=ap.tensor, offset=ap.offset, ap=[[0, 128]] + [list(d) for d in ap.ap[1:]])


# ----------------------------------------------------------------- launch A
def _build_kernel_a(affine):
    nc = bacc.Bacc("TRN2", target_bir_lowering=False, debug=False, num_devices=NC)
    x_dT = nc.dram_tensor("x_dT", [D, TL], F32, kind="ExternalInput")
    rwg9_T = nc.dram_tensor("rwg9_T", [D, 9], F32, kind="ExternalInput")
    c1T = nc.dram_tensor("c1T", [NC, 1], F32, kind="ExternalInput")
    c0T = nc.dram_tensor("c0T", [NC, 1], F32, kind="ExternalInput")
    sgupT = nc.dram_tensor("sgupT", [D, 2 * SDFF], BF16, kind="ExternalInput")
    sdownT = nc.dram_tensor("sdownT", [SDFF, D], BF16, kind="ExternalInput")
    if affine:
        gamT = nc.dram_tensor("gamT", [D, 1], F32, kind="ExternalInput")
        betT = nc.dram_tensor("betT", [D, 1], F32, kind="ExternalInput")
    zT = nc.dram_tensor("zT", [D, TL], BF16, kind="ExternalOutput")
    logitsT = nc.dram_tensor("logitsT", [NC, TL], F32, kind="ExternalOutput")
    ysT = nc.dram_tensor("ysT", [D, TL], F32, kind="ExternalOutput")

    with tile.TileContext(nc) as tc:
        with tc.tile_pool(name="const", bufs=1) as cp, \
             tc.tile_pool(name="xd", bufs=1) as xdp, \
             tc.tile_pool(name="rows", bufs=2) as rp, \
             tc.tile_pool(name="zt", bufs=2) as zp, \
             tc.tile_pool(name="swts", bufs=1) as swp, \
             tc.tile_pool(name="h", bufs=1) as hp, \
             tc.tile_pool(name="ev", bufs=4) as ev_p, \
             tc.tile_pool(name="praw", bufs=1, space="PSUM") as pp, \
             tc.tile_pool(name="ssq", bufs=1, space="PSUM") as qp, \
             tc.tile_pool(name="pg", bufs=2, space="PSUM") as pg_p, \
             tc.tile_pool(name="py", bufs=2, space="PSUM") as py_p:
            # constants
            c1_sb = cp.tile([NC, 1], F32)
            c0_sb = cp.tile([NC, 1], F32)
            nc.gpsimd.dma_start(out=c1_sb[:], in_=c1T[:])
            nc.gpsimd.dma_start(out=c0_sb[:], in_=c0T[:])
            ones_bf = cp.tile([128, 1], BF16)
            nc.vector.memset(ones_bf[:], 1.0)
            ones_row = cp.tile([1, 128], F32)
            nc.vector.memset(ones_row[:], 1.0)
            rwg_sb = cp.tile([128, 8, 9], F32)
            nc.sync.dma_start(out=rwg_sb[:], in_=rwg9_T.rearrange("(k p) e -> p k e", p=128))
            if affine:
                gam_sb = cp.tile([128, 8, 1], F32)
                bet_sb = cp.tile([128, 8, 1], F32)
                nc.gpsimd.dma_start(out=gam_sb[:], in_=gamT.rearrange("(k p) o -> p k o", p=128))
                nc.gpsimd.dma_start(out=bet_sb[:], in_=betT.rearrange("(k p) o -> p k o", p=128))

            xd_sb = xdp.tile([128, 8, TL], F32)
            x2_sb = xdp.tile([128, 8, TL], BF16)
            zT_sb = xdp.tile([128, 8, TL], BF16)
            bc_rstd = xdp.tile([128, TL], F32)
            bc_mrs = xdp.tile([128, TL], F32)
            xd_r = x_dT.rearrange("(k p) t -> p k t", p=128)
            zT_r = zT.rearrange("(k p) t -> p k t", p=128)
            # x_dT in token-column chunks (unblocks router matmuls early)
            for tt in range(8):
                sl = slice(tt * 128, (tt + 1) * 128)
                nc.sync.dma_start(out=xd_sb[:, :, sl], in_=xd_r[:, :, sl])
            # shared expert weights behind x on the sync queue
            sgup_sb = []
            for k in range(8):
                g = swp.tile([128, 2 * SDFF], BF16, tag=f"sgup{k}")
                nc.sync.dma_start(out=g[:], in_=sgupT[k * 128:(k + 1) * 128, :])
                sgup_sb.append(g)
            sdn_sb = []
            for kf in range(SDFF // 128):
                t = swp.tile([128, D], BF16, tag=f"sdn{kf}")
                nc.sync.dma_start(out=t[:], in_=sdownT[kf * 128:(kf + 1) * 128, :])
                sdn_sb.append(t)

            # x^2 in bf16 (scalar engine), trailing the x DMAs
            for tt in range(8):
                sl = slice(tt * 128, (tt + 1) * 128)
                nc.scalar.activation(out=x2_sb[:, :, sl], in_=xd_sb[:, :, sl], func=AF.Square)

            praw_ps = pp.tile([41, 512], F32, space="PSUM")
            ssq_ps = qp.tile([33, 512], F32, space="PSUM")

            def stats_half(h):
                hsl = slice(h * 512, (h + 1) * 512)
                pb = 32 * h  # matmul PSUM base partition must be 0/32/64
                # praw rows 0..7 = x @ (router*gamma).T ; row 8 = sum_d x (fp32)
                for k in range(8):
                    nc.tensor.matmul(out=praw_ps[pb:pb + 9, :], lhsT=rwg_sb[:, k, :],
                                     rhs=xd_sb[:, k, hsl], start=(k == 0), stop=(k == 7))
                for k in range(8):
                    nc.tensor.matmul(out=ssq_ps[pb:pb + 1, :], lhsT=ones_bf[:],
                                     rhs=x2_sb[:, k, hsl], start=(k == 0), stop=(k == 7))
                # evacuate praw to SBUF (engine accesses must start on a
                # 32-partition boundary; the mu row needs a DMA hop to get
                # from partition 8 down to partition 0)
                praw_sb = rp.tile([9, 512], F32, tag="praw_sb")
                nc.vector.tensor_copy(out=praw_sb[:], in_=praw_ps[pb:pb + 9, :])
                mu_raw = rp.tile([1, 512], F32, tag="mu_raw")
                nc.gpsimd.dma_start(out=mu_raw[:], in_=praw_sb[8:9, :])
                # row stats [1, 512]
                mu = rp.tile([1, 512], F32, tag="mu")
                nc.vector.tensor_scalar_mul(mu[:], mu_raw[:], 1.0 / D)
                musq = rp.tile([1, 512], F32, tag="musq")
                nc.vector.tensor_tensor(out=musq[:], in0=mu[:], in1=mu[:], op=OP.mult)
                var = rp.tile([1, 512], F32, tag="var")
                nc.vector.tensor_scalar(out=var[:], in0=ssq_ps[pb:pb + 1, :], scalar1=1.0 / D,
                                        scalar2=LN_EPS, op0=OP.mult, op1=OP.add)
                nc.vector.tensor_tensor(out=var[:], in0=var[:], in1=musq[:], op=OP.subtract)
                std = rp.tile([1, 512], F32, tag="std")
                nc.scalar.activation(out=std[:], in_=var[:], func=AF.Sqrt)
                rstd = rp.tile([1, 512], F32, tag="rstd")
                nc.vector.reciprocal(out=rstd[:], in_=std[:])
                mrs = rp.tile([1, 512], F32, tag="mrs")
                nc.vector.tensor_tensor(out=mrs[:], in0=mu[:], in1=rstd[:], op=OP.mult)
                # broadcast to 128 partitions via PE outer product
                bc1 = pg_p.tile([128, 512], F32, space="PSUM", tag="pg")
                nc.tensor.matmul(out=bc1[:], lhsT=ones_row[:], rhs=rstd[:], start=True, stop=True)
                bc2 = pg_p.tile([128, 512], F32, space="PSUM", tag="pu")
                nc.tensor.matmul(out=bc2[:], lhsT=ones_row[:], rhs=mrs[:], start=True, stop=True)
                nc.vector.tensor_copy(out=bc_rstd[:, hsl], in_=bc1[:])
                nc.vector.tensor_copy(out=bc_mrs[:, hsl], in_=bc2[:])
                # logits = praw*rstd - mrs*c1 + c0   (E-major rows)
                lg = rp.tile([NC, 512], F32, tag="lg")
                nc.vector.tensor_tensor(out=lg[:], in0=praw_sb[0:8, :],
                                        in1=bc1[0:NC, :], op=OP.mult)
                lg2 = rp.tile([NC, 512], F32, tag="lg2")
                nc.vector.tensor_scalar(out=lg2[:], in0=bc2[0:NC, :], scalar1=c1_sb[:],
                                        scalar2=None, op0=OP.mult)
                nc.vector.tensor_tensor(out=lg[:], in0=lg[:], in1=lg2[:], op=OP.subtract)
                nc.vector.tensor_scalar(out=lg[:], in0=lg[:], scalar1=c0_sb[:],
                                        scalar2=None, op0=OP.add)
                nc.scalar.dma_start(out=logitsT[:, hsl], in_=lg[:])
                # zT = x*rstd - mrs  (gpsimd + DVE split), bf16 out
                for k in range(8):
                    t = zp.tile([128, 512], F32, tag=f"zt{k % 2}")
                    nc.gpsimd.tensor_tensor(out=t[:], in0=xd_sb[:, k, hsl], in1=bc_rstd[:, hsl], op=OP.mult)
                    if affine:
                        zf = zp.tile([128, 512], F32, tag=f"zf{k % 2}")
                        nc.vector.tensor_tensor(out=zf[:], in0=t[:], in1=bc_mrs[:, hsl], op=OP.subtract)
                        nc.vector.tensor_scalar(out=zT_sb[:, k, hsl], in0=zf[:], scalar1=gam_sb[:, k, :],
                                                scalar2=bet_sb[:, k, :], op0=OP.mult, op1=OP.add)
                    else:
                        nc.vector.tensor_tensor(out=zT_sb[:, k, hsl], in0=t[:], in1=bc_mrs[:, hsl], op=OP.subtract)
                nc.scalar.dma_start(out=zT_r[:, :, hsl], in_=zT_sb[:, :, hsl])

            def shared_block(b):
                off = b * 512
                hts = []
                for fi in range(SDFF // 128):
                    pg = pg_p.tile([128, 512], F32, space="PSUM", tag="pg")
                    pu = pg_p.tile([128, 512], F32, space="PSUM", tag="pu")
                    for k in range(8):
                        nc.tensor.matmul(out=pg[:], lhsT=sgup_sb[k][:, fi * 128:(fi + 1) * 128],
                                         rhs=zT_sb[:, k, off:off + 512], start=(k == 0), stop=(k == 7))
                    for k in range(8):
                        nc.tensor.matmul(out=pu[:], lhsT=sgup_sb[k][:, (SDFF // 128 + fi) * 128:(SDFF // 128 + fi + 1) * 128],
                                         rhs=zT_sb[:, k, off:off + 512], start=(k == 0), stop=(k == 7))
                    sil = ev_p.tile([128, 512], BF16, tag="sil")
                    nc.scalar.activation(out=sil[:], in_=pg[:], func=AF.Silu)
                    ht = hp.tile([128, 512], BF16, tag=f"sh{fi}")
                    nc.vector.tensor_tensor(out=ht[:], in0=sil[:], in1=pu[:], op=OP.mult)
                    hts.append(ht)
                for m in range(8):
                    py = py_p.tile([128, 512], F32, space="PSUM", tag="py")
                    for kf in range(SDFF // 128):
                        nc.tensor.matmul(out=py[:], lhsT=sdn_sb[kf][:, m * 128:(m + 1) * 128],
                                         rhs=hts[kf][:], start=(kf == 0), stop=(kf == SDFF // 128 - 1))
                    yo = ev_p.tile([128, 512], F32, tag="yo")
                    nc.vector.tensor_copy(out=yo[:], in_=py[:])
                    nc.scalar.dma_start(out=ysT[m * 128:(m + 1) * 128, off:off + 512], in_=yo[:])

            stats_half(0)
            stats_half(1)
            shared_block(0)
            shared_block(1)
    nc.compile()
    return nc


# ----------------------------------------------------------------- launch B
def _build_kernel_b(cap, bw, nb):
    nc = bacc.Bacc("TRN2", target_bir_lowering=False, debug=False, num_devices=NC)
    xcT = nc.dram_tensor("xcT", [D, cap], BF16, kind="ExternalInput")
    gupT = nc.dram_tensor("gupT", [D, 2 * DFF], BF16, kind="ExternalInput")
    downT = nc.dram_tensor("downT", [DFF, D], BF16, kind="ExternalInput")
    gates = nc.dram_tensor("gates", [1, cap], F32, kind="ExternalInput")
    yT = nc.dram_tensor("yT", [D, cap], F32, kind="ExternalOutput")

    nf = DFF // 128
    with tile.TileContext(nc) as tc:
        with tc.tile_pool(name="h", bufs=1) as hp, \
             tc.tile_pool(name="xb", bufs=2) as xp, \
             tc.tile_pool(name="pg", bufs=3, space="PSUM") as pg_p, \
             tc.tile_pool(name="py", bufs=2, space="PSUM") as py_p, \
             tc.tile_pool(name="ev", bufs=4) as ev_p, \
             tc.tile_pool(name="wts", bufs=1) as wtp:
            gat_sb = wtp.tile([128, cap], F32, tag="gat")
            nc.gpsimd.dma_start(out=gat_sb[:], in_=_bc128(gates[:]))
            gup_sb = [wtp.tile([128, 2 * DFF], BF16, tag=f"gup{k}", name=f"gup{k}")
                      for k in range(8)]
            # column chunks interleaved gate/up halves to match per-fi
            # consumption order (pg uses cols fi*128, pu uses cols DFF+fi*128)
            for h in (0, 4, 1, 5, 2, 6, 3, 7):
                for k in range(8):
                    nc.sync.dma_start(
                        out=gup_sb[k][:, h * DFF // 4:(h + 1) * DFF // 4],
                        in_=gupT[k * 128:(k + 1) * 128, h * DFF // 4:(h + 1) * DFF // 4])
            down_sb = []
            for kf in range(nf):
                t = wtp.tile([128, D], BF16, tag=f"dn{kf}")
                nc.sync.dma_start(out=t[:], in_=downT[kf * 128:(kf + 1) * 128, :])
                down_sb.append(t)
            off = 0
            for b in range(nb):
                xT_sb = []
                for k in range(8):
                    x = xp.tile([128, bw], BF16, tag=f"xb{k}")
                    nc.gpsimd.dma_start(out=x[:], in_=xcT[k * 128:(k + 1) * 128, off:off + bw])
                    xT_sb.append(x)
                hts = []
                for fi in range(nf):
                    pg = pg_p.tile([128, bw], F32, space="PSUM", tag="pg")
                    pu = pg_p.tile([128, bw], F32, space="PSUM", tag="pu")
                    for k in range(8):
                        nc.tensor.matmul(out=pg[:], lhsT=gup_sb[k][:, fi * 128:(fi + 1) * 128],
                                         rhs=xT_sb[k][:], start=(k == 0), stop=(k == 7))
                    for k in range(8):
                        nc.tensor.matmul(out=pu[:], lhsT=gup_sb[k][:, (nf + fi) * 128:(nf + fi + 1) * 128],
                                         rhs=xT_sb[k][:], start=(k == 0), stop=(k == 7))
                    sil = ev_p.tile([128, bw], BF16, tag="sil")
                    nc.scalar.activation(out=sil[:], in_=pg[:], func=AF.Silu)
                    ht = hp.tile([128, bw], BF16, tag=f"h{fi}")
                    nc.vector.tensor_tensor(out=ht[:], in0=sil[:], in1=pu[:], op=OP.mult)
                    hts.append(ht)
                for m in range(8):
                    py = py_p.tile([128, bw], F32, space="PSUM", tag="py")
                    for kf in range(nf):
                        nc.tensor.matmul(out=py[:], lhsT=down_sb[kf][:, m * 128:(m + 1) * 128],
                                         rhs=hts[kf][:], start=(kf == 0), stop=(kf == nf - 1))
                    yo = ev_p.tile([128, bw], F32, tag="yo")
                    nc.vector.tensor_tensor(out=yo[:], in0=py[:], in1=gat_sb[:, off:off + bw], op=OP.mult)
                    nc.sync.dma_start(out=yT[m * 128:(m + 1) * 128, off:off + bw], in_=yo[:])
                off += bw
    nc.compile()
    return nc


def _get(key, builder, *args):
    if key not in _CACHE:
        _CACHE[key] = builder(*args)
    return _CACHE[key]


def _to_bf16(a):
    return np.ascontiguousarray(a.astype(ml_dtypes.bfloat16))


def kernel(x, ln_gamma, ln_beta, router_w, gate_up_w, down_w,
           shared_gate_up_w, shared_down_w, _profile=None):
    x = np.asarray(x, np.float32)
    B, S, _ = x.shape
    xt = np.ascontiguousarray(x.reshape(T, D))
    ln_gamma = np.asarray(ln_gamma, np.float32)
    ln_beta = np.asarray(ln_beta, np.float32)
    affine = bool(np.any(ln_gamma != 1.0) or np.any(ln_beta != 0.0))
    rwg9 = np.empty((D, 9), np.float32)
    rwg9[:, :8] = (router_w * ln_gamma[None, :]).T
    rwg9[:, 8] = 1.0
    c1T = (router_w @ ln_gamma).astype(np.float32).reshape(NC, 1)
    c0T = (router_w @ ln_beta).astype(np.float32).reshape(NC, 1)
    sgupT = _to_bf16(shared_gate_up_w.T)
    sdownT = _to_bf16(shared_down_w.T)

    # ---- launch A: LayerNorm + router logits + shared expert (device)
    nc_a = _get(("a", affine), _build_kernel_a, affine)
    in_maps = []
    for c in range(NC):
        sh = xt[c * TL:(c + 1) * TL]
        m = dict(
            x_dT=np.ascontiguousarray(sh.T),
            rwg9_T=rwg9, c1T=c1T, c0T=c0T,
            sgupT=sgupT, sdownT=sdownT,
        )
        if affine:
            m["gamT"] = ln_gamma.reshape(D, 1)
            m["betT"] = ln_beta.reshape(D, 1)
        in_maps.append(m)
    kw = {k: v for k, v in (_profile or {}).items() if k in ("trace", "tmpdir")}
    kwa = dict(kw)
    if "tmpdir" in kwa:
        kwa["tmpdir"] = kwa["tmpdir"] + "_a"
    res_a = run_bass_kernel_spmd(nc_a, in_maps, list(range(NC)), **kwa)
    zT_all = np.concatenate([res_a.results[c]["zT"] for c in range(NC)], axis=1)  # [D, T] bf16
    logits = np.concatenate([res_a.results[c]["logitsT"].T for c in range(NC)], axis=0)
    if _profile is not None:
        _profile["exec_a"] = res_a.exec_time_ns

    # ---- host control plane: softmax / top-2 / capacity compaction
    lg = logits.astype(np.float32)
    p = np.exp(lg - lg.max(-1, keepdims=True))
    p /= p.sum(-1, keepdims=True)
    order = np.argsort(-p, axis=-1, kind="stable")
    top2 = order[:, :2]
    pv = np.take_along_axis(p, top2, axis=1)
    g = np.exp(pv - pv.max(-1, keepdims=True))
    g /= g.sum(-1, keepdims=True)

    idxs, gvals = [], []
    for e in range(NC):
        hit = (top2 == e)
        ide = np.where(hit.any(axis=1))[0]
        ge = np.where(hit[ide, 0], g[ide, 0], g[ide, 1]).astype(np.float32)
        idxs.append(ide)
        gvals.append(ge)
    max_load = max(len(ide) for ide in idxs)
    nb = max(1, math.ceil(max_load / 512))
    bw = math.ceil(max_load / nb / 16) * 16
    cap = nb * bw

    # ---- launch B: routed expert FFNs (device, expert-parallel)
    nc_b = _get(("b", cap, bw, nb), _build_kernel_b, cap, bw, nb)
    in_maps = []
    for c in range(NC):
        ide, ge = idxs[c], gvals[c]
        xc = np.zeros((D, cap), ml_dtypes.bfloat16)
        xc[:, :len(ide)] = zT_all[:, ide]
        gr = np.zeros((1, cap), np.float32)
        gr[0, :len(ide)] = ge
        in_maps.append(dict(
            xcT=xc,
            gupT=_to_bf16(gate_up_w[c].T),
            downT=_to_bf16(down_w[c].T),
            gates=gr,
        ))
    kwb = dict(kw)
    if "tmpdir" in kwb:
        kwb["tmpdir"] = kwb["tmpdir"] + "_b"
    res_b = run_bass_kernel_spmd(nc_b, in_maps, list(range(NC)), **kwb)
    if _profile is not None:
        _profile["exec_b"] = res_b.exec_time_ns

    # ---- host: scatter-add combine (data movement + elementwise add)
    out = np.zeros((T, D), np.float32)
    for c in range(NC):
        ide = idxs[c]
        out[ide] += res_b.results[c]["yT"][:, :len(ide)].T
        out[c * TL:(c + 1) * TL] += res_a.results[c]["ysT"].T
    return out.reshape(B, S, D)


# revision 15
# speedup vs baseline: 1.1016x; 1.0288x over previous
"""MoE FeedForward kernel for Trainium2 (8 NeuronCores).

Strategy:
  - Launch A (data-parallel over tokens): each core computes router logits
    (fp32, exact, E-major with a fused ones-column for the token mean),
    sum-of-squares via a bf16 ones-matmul, per-token LayerNorm stats as
    [1, T] rows, broadcasts them with PE outer products, produces the
    normalized tokens DIRECTLY in transposed (d-major) layout via DVE, and
    runs the SHARED expert FFN over its shard (filling the otherwise-idle
    tensor engine).
  - Host control plane: softmax/top-2/gate weights + capacity-padded token
    compaction per expert (integer bookkeeping + data shuffling only).
  - Launch B (expert-parallel): core c holds expert c's weights, runs the
    routed SwiGLU FFN over its compacted tokens in bf16 (fp32 accumulate)
    and applies the combine gate on device. Capacity is derived from the
    actual max expert load at runtime (compiled per capacity, cached).
    Host scatters the gated expert outputs back and sums.
"""

import math

import numpy as np
import ml_dtypes

import concourse.bass as bass
import concourse.mybir as mybir
import concourse.tile as tile
from concourse import bacc
from concourse.bass_utils import run_bass_kernel_spmd

F32 = mybir.dt.float32
BF16 = mybir.dt.bfloat16
AF = mybir.ActivationFunctionType
OP = mybir.AluOpType
AX = mybir.AxisListType

NC = 8          # cores / experts
D = 1024        # d_model
DFF = 3072      # routed expert ffn dim
SDFF = 1024     # shared expert ffn dim
T = 8192        # total tokens
TL = T // NC    # tokens per core (launch A)
LN_EPS = 1e-5

_CACHE = {}


def _bc128(ap):
    """Broadcast a [1, N] DRAM AP across 128 partitions (0-step partition dim)."""
    return bass.AP(tensor=ap.tensor, offset=ap.offset, ap=[[0, 128]] + [list(d) for d in ap.ap[1:]])


# ----------------------------------------------------------------- launch A
def _build_kernel_a(affine):
    nc = bacc.Bacc("TRN2", target_bir_lowering=False, debug=False, num_devices=NC)
    x_dT = nc.dram_tensor("x_dT", [D, TL], F32, kind="ExternalInput")
    rwg9_T = nc.dram_tensor("rwg9_T", [D, 9], F32, kind="ExternalInput")
    c1T = nc.dram_tensor("c1T", [NC, 1], F32, kind="ExternalInput")
    c0T = nc.dram_tensor("c0T", [NC, 1], F32, kind="ExternalInput")
    sgupT = nc.dram_tensor("sgupT", [D, 2 * SDFF], BF16, kind="ExternalInput")
    sdownT = nc.dram_tensor("sdownT", [SDFF, D], BF16, kind="ExternalInput")
    if affine:
        gamT = nc.dram_tensor("gamT", [D, 1], F32, kind="ExternalInput")
        betT = nc.dram_tensor("betT", [D, 1], F32, kind="ExternalInput")
    zT = nc.dram_tensor("zT", [D, TL], BF16, kind="ExternalOutput")
    logitsT = nc.dram_tensor("logitsT", [NC, TL], F32, kind="ExternalOutput")
    ysT = nc.dram_tensor("ysT", [D, TL], F32, kind="ExternalOutput")

    with tile.TileContext(nc) as tc:
        with tc.tile_pool(name="const", bufs=1) as cp, \
             tc.tile_pool(name="xd", bufs=1) as xdp, \
             tc.tile_pool(name="rows", bufs=2) as rp, \
             tc.tile_pool(name="zt", bufs=2) as zp, \
             tc.tile_pool(name="swts", bufs=1) as swp, \
             tc.tile_pool(name="h", bufs=1) as hp, \
             tc.tile_pool(name="ev", bufs=4) as ev_p, \
             tc.tile_pool(name="praw", bufs=1, space="PSUM") as pp, \
             tc.tile_pool(name="ssq", bufs=1, space="PSUM") as qp, \
             tc.tile_pool(name="pg", bufs=2, space="PSUM") as pg_p, \
             tc.tile_pool(name="py", bufs=2, space="PSUM") as py_p:
            # constants
            c1_sb = cp.tile([NC, 1], F32)
            c0_sb = cp.tile([NC, 1], F32)
            nc.gpsimd.dma_start(out=c1_sb[:], in_=c1T[:])
            nc.gpsimd.dma_start(out=c0_sb[:], in_=c0T[:])
            ones_bf = cp.tile([128, 1], BF16)
            nc.vector.memset(ones_bf[:], 1.0)
            ones_row = cp.tile([1, 128], F32)
            nc.vector.memset(ones_row[:], 1.0)
            rwg_sb = cp.tile([128, 8, 9], F32)
            nc.sync.dma_start(out=rwg_sb[:], in_=rwg9_T.rearrange("(k p) e -> p k e", p=128))
            if affine:
                gam_sb = cp.tile([128, 8, 1], F32)
                bet_sb = cp.tile([128, 8, 1], F32)
                nc.gpsimd.dma_start(out=gam_sb[:], in_=gamT.rearrange("(k p) o -> p k o", p=128))
                nc.gpsimd.dma_start(out=bet_sb[:], in_=betT.rearrange("(k p) o -> p k o", p=128))

            xd_sb = xdp.tile([128, 8, TL], F32)
            x2_sb = xdp.tile([128, 8, TL], BF16)
            zT_sb = xdp.tile([128, 8, TL], BF16)
            bc_rstd = xdp.tile([128, TL], F32)
            bc_mrs = xdp.tile([128, TL], F32)
            xd_r = x_dT.rearrange("(k p) t -> p k t", p=128)
            zT_r = zT.rearrange("(k p) t -> p k t", p=128)
            # x_dT per (half, k) chunk on sync: first router matmul unblocks
            # after one 256KB transfer
            for h in range(2):
                hsl = slice(h * 512, (h + 1) * 512)
                for k in range(8):
                    nc.sync.dma_start(out=xd_sb[:, k, hsl], in_=xd_r[:, k, hsl])
            # shared expert weights on the gpsimd queue (parallel with x)
            sgup_sb = []
            for k in range(8):
                g = swp.tile([128, 2 * SDFF], BF16, tag=f"sgup{k}")
                nc.gpsimd.dma_start(out=g[:], in_=sgupT[k * 128:(k + 1) * 128, :])
                sgup_sb.append(g)
            sdn_sb = []
            for kf in range(SDFF // 128):
                t = swp.tile([128, D], BF16, tag=f"sdn{kf}")
                nc.gpsimd.dma_start(out=t[:], in_=sdownT[kf * 128:(kf + 1) * 128, :])
                sdn_sb.append(t)

            # x^2 in bf16 (scalar engine), trailing the x DMAs
            for h in range(2):
                hsl = slice(h * 512, (h + 1) * 512)
                for k in range(8):
                    nc.scalar.activation(out=x2_sb[:, k, hsl], in_=xd_sb[:, k, hsl], func=AF.Square)

            praw_ps = pp.tile([41, 512], F32, space="PSUM")
            ssq_ps = qp.tile([33, 512], F32, space="PSUM")

            def stats_mm(h):
                hsl = slice(h * 512, (h + 1) * 512)
                pb = 32 * h  # matmul PSUM base partition must be 0/32/64
                # praw rows 0..7 = x @ (router*gamma).T ; row 8 = sum_d x (fp32)
                for k in range(8):
                    nc.tensor.matmul(out=praw_ps[pb:pb + 9, :], lhsT=rwg_sb[:, k, :],
                                     rhs=xd_sb[:, k, hsl], start=(k == 0), stop=(k == 7))
                for k in range(8):
                    nc.tensor.matmul(out=ssq_ps[pb:pb + 1, :], lhsT=ones_bf[:],
                                     rhs=x2_sb[:, k, hsl], start=(k == 0), stop=(k == 7))

            def stats_rows(h):
                hsl = slice(h * 512, (h + 1) * 512)
                pb = 32 * h
                # evacuate praw to SBUF (engine accesses must start on a
                # 32-partition boundary; the mu row needs a DMA hop to get
                # from partition 8 down to partition 0)
                praw_sb = rp.tile([9, 512], F32, tag="praw_sb")
                nc.vector.tensor_copy(out=praw_sb[:], in_=praw_ps[pb:pb + 9, :])
                mu_raw = rp.tile([1, 512], F32, tag="mu_raw")
                nc.scalar.dma_start(out=mu_raw[:], in_=praw_sb[8:9, :])
                # row stats [1, 512]
                mu = rp.tile([1, 512], F32, tag="mu")
                nc.vector.tensor_scalar_mul(mu[:], mu_raw[:], 1.0 / D)
                musq = rp.tile([1, 512], F32, tag="musq")
                nc.vector.tensor_tensor(out=musq[:], in0=mu[:], in1=mu[:], op=OP.mult)
                var = rp.tile([1, 512], F32, tag="var")
                nc.vector.tensor_scalar(out=var[:], in0=ssq_ps[pb:pb + 1, :], scalar1=1.0 / D,
                                        scalar2=LN_EPS, op0=OP.mult, op1=OP.add)
                nc.vector.tensor_tensor(out=var[:], in0=var[:], in1=musq[:], op=OP.subtract)
                std = rp.tile([1, 512], F32, tag="std")
                nc.scalar.activation(out=std[:], in_=var[:], func=AF.Sqrt)
                rstd = rp.tile([1, 512], F32, tag="rstd")
                nc.vector.reciprocal(out=rstd[:], in_=std[:])
                mrs = rp.tile([1, 512], F32, tag="mrs")
                nc.vector.tensor_tensor(out=mrs[:], in0=mu[:], in1=rstd[:], op=OP.mult)
                # broadcast to 128 partitions via PE outer product
                bc1 = pg_p.tile([128, 512], F32, space="PSUM", tag="pg")
                nc.tensor.matmul(out=bc1[:], lhsT=ones_row[:], rhs=rstd[:], start=True, stop=True)
                bc2 = pg_p.tile([128, 512], F32, space="PSUM", tag="pu")
                nc.tensor.matmul(out=bc2[:], lhsT=ones_row[:], rhs=mrs[:], start=True, stop=True)
                nc.vector.tensor_copy(out=bc_rstd[:, hsl], in_=bc1[:])
                nc.vector.tensor_copy(out=bc_mrs[:, hsl], in_=bc2[:])
                # logits = praw*rstd - mrs*c1 + c0   (E-major rows)
                lg = rp.tile([NC, 512], F32, tag="lg")
                nc.vector.tensor_tensor(out=lg[:], in0=praw_sb[0:8, :],
                                        in1=bc1[0:NC, :], op=OP.mult)
                lg2 = rp.tile([NC, 512], F32, tag="lg2")
                nc.vector.tensor_scalar(out=lg2[:], in0=bc2[0:NC, :], scalar1=c1_sb[:],
                                        scalar2=None, op0=OP.mult)
                nc.vector.tensor_tensor(out=lg[:], in0=lg[:], in1=lg2[:], op=OP.subtract)
                nc.vector.tensor_scalar(out=lg[:], in0=lg[:], scalar1=c0_sb[:],
                                        scalar2=None, op0=OP.add)
                nc.scalar.dma_start(out=logitsT[:, hsl], in_=lg[:])
                # zT = x*rstd - mrs  (DVE), bf16 out
                for k in range(8):
                    t = zp.tile([128, 512], F32, tag=f"zt{k % 2}")
                    nc.vector.tensor_tensor(out=t[:], in0=xd_sb[:, k, hsl], in1=bc_rstd[:, hsl], op=OP.mult)
                    if affine:
                        zf = zp.tile([128, 512], F32, tag=f"zf{k % 2}")
                        nc.vector.tensor_tensor(out=zf[:], in0=t[:], in1=bc_mrs[:, hsl], op=OP.subtract)
                        nc.vector.tensor_scalar(out=zT_sb[:, k, hsl], in0=zf[:], scalar1=gam_sb[:, k, :],
                                                scalar2=bet_sb[:, k, :], op0=OP.mult, op1=OP.add)
                    else:
                        nc.vector.tensor_tensor(out=zT_sb[:, k, hsl], in0=t[:], in1=bc_mrs[:, hsl], op=OP.subtract)
                nc.scalar.dma_start(out=zT_r[:, :, hsl], in_=zT_sb[:, :, hsl])

            def shared_block(b):
                off = b * 512
                hts = []
                for fi in range(SDFF // 128):
                    pg = pg_p.tile([128, 512], F32, space="PSUM", tag="pg")
                    pu = pg_p.tile([128, 512], F32, space="PSUM", tag="pu")
                    for k in range(8):
                        nc.tensor.matmul(out=pg[:], lhsT=sgup_sb[k][:, fi * 128:(fi + 1) * 128],
                                         rhs=zT_sb[:, k, off:off + 512], start=(k == 0), stop=(k == 7))
                    for k in range(8):
                        nc.tensor.matmul(out=pu[:], lhsT=sgup_sb[k][:, (SDFF // 128 + fi) * 128:(SDFF // 128 + fi + 1) * 128],
                                         rhs=zT_sb[:, k, off:off + 512], start=(k == 0), stop=(k == 7))
                    sil = ev_p.tile([128, 512], BF16, tag="sil")
                    nc.scalar.activation(out=sil[:], in_=pg[:], func=AF.Silu)
                    ht = hp.tile([128, 512], BF16, tag=f"sh{fi}")
                    nc.vector.tensor_tensor(out=ht[:], in0=sil[:], in1=pu[:], op=OP.mult)
                    hts.append(ht)
                for m in range(8):
                    py = py_p.tile([128, 512], F32, space="PSUM", tag="py")
                    for kf in range(SDFF // 128):
                        nc.tensor.matmul(out=py[:], lhsT=sdn_sb[kf][:, m * 128:(m + 1) * 128],
                                         rhs=hts[kf][:], start=(kf == 0), stop=(kf == SDFF // 128 - 1))
                    yo = ev_p.tile([128, 512], F32, tag="yo")
                    nc.vector.tensor_copy(out=yo[:], in_=py[:])
                    nc.scalar.dma_start(out=ysT[m * 128:(m + 1) * 128, off:off + 512], in_=yo[:])

            stats_mm(0)
            stats_mm(1)
            stats_rows(0)
            stats_rows(1)
            shared_block(0)
            shared_block(1)
    nc.compile()
    return nc


# ----------------------------------------------------------------- launch B
def _build_kernel_b(cap, bw, nb):
    nc = bacc.Bacc("TRN2", target_bir_lowering=False, debug=False, num_devices=NC)
    xcT = nc.dram_tensor("xcT", [D, cap], BF16, kind="ExternalInput")
    gupT = nc.dram_tensor("gupT", [D, 2 * DFF], BF16, kind="ExternalInput")
    downT = nc.dram_tensor("downT", [DFF, D], BF16, kind="ExternalInput")
    gates = nc.dram_tensor("gates", [1, cap], F32, kind="ExternalInput")
    yT = nc.dram_tensor("yT", [D, cap], F32, kind="ExternalOutput")

    nf = DFF // 128
    with tile.TileContext(nc) as tc:
        with tc.tile_pool(name="h", bufs=1) as hp, \
             tc.tile_pool(name="xb", bufs=2) as xp, \
             tc.tile_pool(name="pg", bufs=3, space="PSUM") as pg_p, \
             tc.tile_pool(name="py", bufs=2, space="PSUM") as py_p, \
             tc.tile_pool(name="ev", bufs=4) as ev_p, \
             tc.tile_pool(name="wts", bufs=1) as wtp:
            gat_sb = wtp.tile([128, cap], F32, tag="gat")
            nc.scalar.dma_start(out=gat_sb[:], in_=_bc128(gates[:]))
            gup_sb = [wtp.tile([128, 2 * DFF], BF16, tag=f"gup{k}", name=f"gup{k}")
                      for k in range(8)]
            # column chunks interleaved gate/up halves to match per-fi
            # consumption order (pg uses cols fi*128, pu uses cols DFF+fi*128)
            for h in (0, 4, 1, 5, 2, 6, 3, 7):
                for k in range(8):
                    nc.sync.dma_start(
                        out=gup_sb[k][:, h * DFF // 4:(h + 1) * DFF // 4],
                        in_=gupT[k * 128:(k + 1) * 128, h * DFF // 4:(h + 1) * DFF // 4])
            down_sb = []
            for kf in range(nf):
                t = wtp.tile([128, D], BF16, tag=f"dn{kf}")
                nc.sync.dma_start(out=t[:], in_=downT[kf * 128:(kf + 1) * 128, :])
                down_sb.append(t)
            off = 0
            for b in range(nb):
                xT_sb = []
                for k in range(8):
                    x = xp.tile([128, bw], BF16, tag=f"xb{k}")
                    nc.gpsimd.dma_start(out=x[:], in_=xcT[k * 128:(k + 1) * 128, off:off + bw])
                    xT_sb.append(x)
                hts = []
                for fi in range(nf):
                    pg = pg_p.tile([128, bw], F32, space="PSUM", tag="pg")
                    pu = pg_p.tile([128, bw], F32, space="PSUM", tag="pu")
                    for k in range(8):
                        nc.tensor.matmul(out=pg[:], lhsT=gup_sb[k][:, fi * 128:(fi + 1) * 128],
                                         rhs=xT_sb[k][:], start=(k == 0), stop=(k == 7))
                    for k in range(8):
                        nc.tensor.matmul(out=pu[:], lhsT=gup_sb[k][:, (nf + fi) * 128:(nf + fi + 1) * 128],
                                         rhs=xT_sb[k][:], start=(k == 0), stop=(k == 7))
                    sil = ev_p.tile([128, bw], BF16, tag="sil")
                    nc.scalar.activation(out=sil[:], in_=pg[:], func=AF.Silu)
                    ht = hp.tile([128, bw], BF16, tag=f"h{fi}")
                    nc.vector.tensor_tensor(out=ht[:], in0=sil[:], in1=pu[:], op=OP.mult)
                    hts.append(ht)
                for m in range(8):
                    py = py_p.tile([128, bw], F32, space="PSUM", tag="py")
                    for kf in range(nf):
                        nc.tensor.matmul(out=py[:], lhsT=down_sb[kf][:, m * 128:(m + 1) * 128],
                                         rhs=hts[kf][:], start=(kf == 0), stop=(kf == nf - 1))
                    yo = ev_p.tile([128, bw], F32, tag="yo")
                    nc.vector.tensor_tensor(out=yo[:], in0=py[:], in1=gat_sb[:, off:off + bw], op=OP.mult)
                    nc.sync.dma_start(out=yT[m * 128:(m + 1) * 128, off:off + bw], in_=yo[:])
                off += bw
    nc.compile()
    return nc


def _get(key, builder, *args):
    if key not in _CACHE:
        _CACHE[key] = builder(*args)
    return _CACHE[key]


def _to_bf16(a):
    return np.ascontiguousarray(a.astype(ml_dtypes.bfloat16))


def kernel(x, ln_gamma, ln_beta, router_w, gate_up_w, down_w,
           shared_gate_up_w, shared_down_w, _profile=None):
    x = np.asarray(x, np.float32)
    B, S, _ = x.shape
    xt = np.ascontiguousarray(x.reshape(T, D))
    ln_gamma = np.asarray(ln_gamma, np.float32)
    ln_beta = np.asarray(ln_beta, np.float32)
    affine = bool(np.any(ln_gamma != 1.0) or np.any(ln_beta != 0.0))
    rwg9 = np.empty((D, 9), np.float32)
    rwg9[:, :8] = (router_w * ln_gamma[None, :]).T
    rwg9[:, 8] = 1.0
    c1T = (router_w @ ln_gamma).astype(np.float32).reshape(NC, 1)
    c0T = (router_w @ ln_beta).astype(np.float32).reshape(NC, 1)
    sgupT = _to_bf16(shared_gate_up_w.T)
    sdownT = _to_bf16(shared_down_w.T)

    # ---- launch A: LayerNorm + router logits + shared expert (device)
    nc_a = _get(("a", affine), _build_kernel_a, affine)
    in_maps = []
    for c in range(NC):
        sh = xt[c * TL:(c + 1) * TL]
        m = dict(
            x_dT=np.ascontiguousarray(sh.T),
            rwg9_T=rwg9, c1T=c1T, c0T=c0T,
            sgupT=sgupT, sdownT=sdownT,
        )
        if affine:
            m["gamT"] = ln_gamma.reshape(D, 1)
            m["betT"] = ln_beta.reshape(D, 1)
        in_maps.append(m)
    kw = {k: v for k, v in (_profile or {}).items() if k in ("trace", "tmpdir")}
    kwa = dict(kw)
    if "tmpdir" in kwa:
        kwa["tmpdir"] = kwa["tmpdir"] + "_a"
    res_a = run_bass_kernel_spmd(nc_a, in_maps, list(range(NC)), **kwa)
    zT_all = np.concatenate([res_a.results[c]["zT"] for c in range(NC)], axis=1)  # [D, T] bf16
    logits = np.concatenate([res_a.results[c]["logitsT"].T for c in range(NC)], axis=0)
    if _profile is not None:
        _profile["exec_a"] = res_a.exec_time_ns

    # ---- host control plane: softmax / top-2 / capacity compaction
    lg = logits.astype(np.float32)
    p = np.exp(lg - lg.max(-1, keepdims=True))
    p /= p.sum(-1, keepdims=True)
    order = np.argsort(-p, axis=-1, kind="stable")
    top2 = order[:, :2]
    pv = np.take_along_axis(p, top2, axis=1)
    g = np.exp(pv - pv.max(-1, keepdims=True))
    g /= g.sum(-1, keepdims=True)

    idxs, gvals = [], []
    for e in range(NC):
        hit = (top2 == e)
        ide = np.where(hit.any(axis=1))[0]
        ge = np.where(hit[ide, 0], g[ide, 0], g[ide, 1]).astype(np.float32)
        idxs.append(ide)
        gvals.append(ge)
    max_load = max(len(ide) for ide in idxs)
    nb = max(1, math.ceil(max_load / 512))
    bw = math.ceil(max_load / nb / 16) * 16
    cap = nb * bw

    # ---- launch B: routed expert FFNs (device, expert-parallel)
    nc_b = _get(("b", cap, bw, nb), _build_kernel_b, cap, bw, nb)
    in_maps = []
    for c in range(NC):
        ide, ge = idxs[c], gvals[c]
        xc = np.zeros((D, cap), ml_dtypes.bfloat16)
        xc[:, :len(ide)] = zT_all[:, ide]
        gr = np.zeros((1, cap), np.float32)
        gr[0, :len(ide)] = ge
        in_maps.append(dict(
            xcT=xc,
            gupT=_to_bf16(gate_up_w[c].T),
            downT=_to_bf16(down_w[c].T),
            gates=gr,
        ))
    kwb = dict(kw)
    if "tmpdir" in kwb:
        kwb["tmpdir"] = kwb["tmpdir"] + "_b"
    res_b = run_bass_kernel_spmd(nc_b, in_maps, list(range(NC)), **kwb)
    if _profile is not None:
        _profile["exec_b"] = res_b.exec_time_ns

    # ---- host: scatter-add combine (data movement + elementwise add)
    out = np.zeros((T, D), np.float32)
    for c in range(NC):
        ide = idxs[c]
        out[ide] += res_b.results[c]["yT"][:, :len(ide)].T
        out[c * TL:(c + 1) * TL] += res_a.results[c]["ysT"].T
    return out.reshape(B, S, D)
